# revision 37
# baseline (speedup 1.0000x reference)
"""Trainium2 Bass kernel for nn_Attention_79164837199973.

Bias-augmented multi-head self-attention with sigmoid gating.
B=4, N=1024, CQ=CH=512, H=8, D=64.

Sharding (8 cores, no collectives): core c -> batch b=c//2, query-row half
r=c%2 (512 rows). Each core computes k/v projections for the full sequence
of its batch (duplicated across the 2 cores of a batch pair -- cheaper than
an all-reduce), attention for all 8 heads over its 512 query rows, then
to_out + gating. Per-core outputs are exact disjoint shards of the result.

Layout notes (all matmuls bf16, f32 PSUM accumulate):
  - every DRAM input is host-pre-swizzled to its exact SBUF layout
    ([128 partitions, ...] with contiguous per-partition bytes) so each
    dma_start lowers to ~16 fat descriptors instead of ~1000 strided ones.
  - scores computed transposed: sT[nk,nq] via matmul(lhsT=kT[d,nk_chunk],
    rhs=qT[d,nq]); K=d=64, two heads row-packed into the PE array
    (partitions 0-63 / 64-127 concurrently).
  - softmax without max-subtraction (logits bounded ~+-7 here):
    p = exp(qk) * exp(bias), exp(bias) precomputed on host (bf16),
    multiplied in on VectorE (bf16 packed mode).
  - denominator via a ones-column appended to V (M=65 PV matmul, row 64);
    reciprocals batched 4 heads at a time into [4,512] DVE reciprocals
    (reciprocal costs ~6.5 cyc per per-lane element regardless of lane
    count, so 8 separate [1,512] ops would cost 8x).
  - per-head 1/denom broadcast over 64 partitions via tiny K=8 PE matmuls
    against a [8, 8*64] one-hot selector.
"""

import os
import sys

sys.path.insert(0, "/opt/trn_rl_repo")

import numpy as np

import concourse.bass as bass
import concourse.tile as tile
from concourse import bacc, mybir

B, N, CQ, CH, H = 4, 1024, 512, 512, 8
D = CH // H  # 64
NQ = N // 2  # 512 query rows per core
P = 128
F32 = mybir.dt.float32
BF16 = mybir.dt.bfloat16
AF = mybir.ActivationFunctionType


def build_nc():
    nc = bacc.Bacc("TRN2", target_bir_lowering=False, debug=False, num_devices=8)

    # ---- DRAM parameters, already in SBUF layout (host pre-swizzled) ----
    xt_e = nc.declare_dram_parameter("xt", [P, 4, N], BF16, isOutput=False)
    xtq_e = nc.declare_dram_parameter("xtq", [P, 4, NQ], BF16, isOutput=False)
    ebt_e = nc.declare_dram_parameter("ebt", [P, H * 8, NQ], BF16, isOutput=False)
    wqt_e = nc.declare_dram_parameter("wqt", [P, 4, CH], BF16, isOutput=False)
    wkt_e = nc.declare_dram_parameter("wkt", [P, 4, CH], BF16, isOutput=False)
    wvt_e = nc.declare_dram_parameter("wvt", [P, 4, CH], BF16, isOutput=False)
    wot_e = nc.declare_dram_parameter("wot", [P, 4, CQ], BF16, isOutput=False)
    wgt_e = nc.declare_dram_parameter("wgt", [P, 4, CQ], BF16, isOutput=False)
    bqs_e = nc.declare_dram_parameter("bqs", [P, 4], F32, isOutput=False)
    bo_e = nc.declare_dram_parameter("bo", [P, 4], F32, isOutput=False)
    gb_e = nc.declare_dram_parameter("gb", [P, 4], F32, isOutput=False)
    sel8_e = nc.declare_dram_parameter("sel8", [H, H * D], BF16, isOutput=False)
    out_e = nc.declare_dram_parameter("out", [P, 4, NQ], F32, isOutput=True)

    with tile.TileContext(nc) as tc:
        with (
            tc.tile_pool(name="singles", bufs=1) as singles,
            tc.tile_pool(name="etmp", bufs=3) as etmp,
            tc.tile_pool(name="vtmp", bufs=2) as vtmp,
            tc.tile_pool(name="ps_s", bufs=2, space="PSUM") as ps_s,
            tc.tile_pool(name="ps_sm", bufs=4, space="PSUM") as ps_sm,
        ):
            # ---- persistent SBUF tiles ----
            xt_sb = singles.tile([P, 4, N], BF16)
            xtq_sb = singles.tile([P, 4, NQ], BF16)
            wqt_sb = singles.tile([P, 4, CH], BF16)
            wkt_sb = singles.tile([P, 4, CH], BF16)
            wvt_sb = singles.tile([P, 4, CH], BF16)
            wot_sb = singles.tile([P, 4, CQ], BF16)
            wgt_sb = singles.tile([P, 4, CQ], BF16)
            bqs_sb = singles.tile([P, 4], F32)
            bo_sb = singles.tile([P, 4], F32)
            gb_sb = singles.tile([P, 4], F32)
            sel8_sb = singles.tile([H, H * D], BF16)
            ebt_sb = singles.tile([P, H * 8, NQ], BF16)
            kt_sb = singles.tile([P, 4, N], BF16)
            qt_sb = singles.tile([P, 4, NQ], BF16)
            vaug_sb = singles.tile([P, 8, H * (D + 1)], BF16)
            oraw_sb = singles.tile([D + 1, H, NQ], BF16)  # 0-63 o, row 64 den
            osc_sb = singles.tile([D, H, NQ], BF16)  # normalized o, all heads
            ofin_sb = singles.tile([P, 4, NQ], BF16)  # head-merged o^T
            z_sb = singles.tile([P, 4, NQ], F32)  # gate pre-activation
            gate_sb = singles.tile([P, 4, NQ], BF16)
            outf_sb = singles.tile([P, 4, NQ], F32)
            warm_sb = singles.tile([1, 8], F32)
            warmo_sb = singles.tile([1, 8], BF16)

            # force the exp table load off the critical path (first ACT op)
            nc.vector.memset(warm_sb, 0.0)
            nc.scalar.activation(out=warmo_sb, in_=warm_sb, func=AF.Exp)

            # ---- input DMAs: one HWDGE ring; FIFO order = priority order
            def ebt_load(h):
                nc.sync.dma_start(
                    out=ebt_sb[:, h * 8 : (h + 1) * 8, :],
                    in_=ebt_e[:, h * 8 : (h + 1) * 8, :],
                )

            nc.sync.dma_start(out=xt_sb, in_=xt_e[:, :, :])
            nc.sync.dma_start(out=wkt_sb, in_=wkt_e[:, :, :])
            nc.sync.dma_start(out=wqt_sb, in_=wqt_e[:, :, :])
            nc.sync.dma_start(out=xtq_sb, in_=xtq_e[:, :, :])
            nc.sync.dma_start(out=bqs_sb, in_=bqs_e[:, :])
            nc.sync.dma_start(out=wvt_sb, in_=wvt_e[:, :, :])
            for h in range(4):
                ebt_load(h)
            nc.sync.dma_start(out=wgt_sb, in_=wgt_e[:, :, :])
            nc.sync.dma_start(out=gb_sb, in_=gb_e[:, :])
            nc.sync.dma_start(out=sel8_sb, in_=sel8_e[:, :])
            for h in range(4, H):
                ebt_load(h)
            nc.sync.dma_start(out=wot_sb, in_=wot_e[:, :, :])
            nc.sync.dma_start(out=bo_sb, in_=bo_e[:, :])

            # ones column of v_aug (col D of each head's 65-wide group)
            nc.vector.memset(
                vaug_sb.rearrange("p c (h e) -> p c h e", h=H)[:, :, :, D : D + 1],
                1.0,
            )

            def kt_proj(mo, act_evac=False):
                for no in range(2):
                    ps = ps_sm.tile([P, 512], F32, tag="ps", name="ps_k")
                    for ko in range(4):
                        nc.tensor.matmul(
                            ps,
                            lhsT=wkt_sb[:, ko, mo * P : (mo + 1) * P],
                            rhs=xt_sb[:, ko, no * 512 : (no + 1) * 512],
                            start=(ko == 0),
                            stop=(ko == 3),
                        )
                    dst = kt_sb[:, mo, no * 512 : (no + 1) * 512]
                    if act_evac:
                        nc.scalar.copy(out=dst, in_=ps)
                    else:
                        nc.vector.tensor_copy(out=dst, in_=ps)

            def qt_proj(mo, act_evac=False):
                ps = ps_sm.tile([P, 512], F32, tag="ps", name="ps_q")
                for ko in range(4):
                    nc.tensor.matmul(
                        ps,
                        lhsT=wqt_sb[:, ko, mo * P : (mo + 1) * P],
                        rhs=xtq_sb[:, ko, :],
                        start=(ko == 0),
                        stop=(ko == 3),
                    )
                if act_evac:
                    nc.scalar.activation(
                        out=qt_sb[:, mo, :],
                        in_=ps,
                        func=AF.Identity,
                        bias=bqs_sb[:, mo : mo + 1],
                    )
                else:
                    nc.vector.tensor_scalar_add(
                        out=qt_sb[:, mo, :], in0=ps, scalar1=bqs_sb[:, mo : mo + 1]
                    )

            def v_proj(c, act_evac=False):
                ps = ps_sm.tile([P, 512], F32, tag="ps", name="ps_v")
                for ko in range(4):
                    nc.tensor.matmul(
                        ps,
                        lhsT=xt_sb[:, ko, c * P : (c + 1) * P],
                        rhs=wvt_sb[:, ko, :],
                        start=(ko == 0),
                        stop=(ko == 3),
                    )
                dst = vaug_sb.rearrange("p c (h e) -> p c h e", h=H)[:, c, :, 0:D]
                srcv = ps.rearrange("p (h d) -> p h d", h=H)
                if act_evac:
                    nc.scalar.copy(out=dst, in_=srcv)
                else:
                    nc.vector.tensor_copy(out=dst, in_=srcv)

            def gate_proj(mo):
                ps = ps_sm.tile([P, 512], F32, tag="ps", name="ps_g")
                for ko in range(4):
                    nc.tensor.matmul(
                        ps,
                        lhsT=wgt_sb[:, ko, mo * P : (mo + 1) * P],
                        rhs=xtq_sb[:, ko, :],
                        start=(ko == 0),
                        stop=(ko == 3),
                    )
                nc.vector.tensor_scalar_add(
                    out=z_sb[:, mo, :], in0=ps, scalar1=gb_sb[:, mo : mo + 1]
                )

            def attention_pair(hp, pre_pv=None, after_first_exp=None):
                heads = (2 * hp, 2 * hp + 1)
                pv_ps = {}
                for h in heads:
                    pv_ps[h] = ps_sm.tile(
                        [D + 1, NQ], F32, tag="ps", name=f"pv_{h}"
                    )
                for t in range(4):  # two nk-chunks of 128 per step
                    s_tiles = {}
                    for h in heads:
                        s_tiles[h] = ps_s.tile(
                            [P, 2, 512], F32, tag="s", name=f"s_{h}_{t}"
                        )
                    # j outer / h inner: the two heads' K=64 matmuls are
                    # adjacent in the PE stream -> row-packed concurrency
                    for j in range(2):
                        c = 2 * t + j
                        for h in heads:
                            d0 = (h % 2) * D
                            mo = h // 2
                            nc.tensor.matmul(
                                s_tiles[h][:, j, :],
                                lhsT=kt_sb[d0 : d0 + D, mo, c * P : (c + 1) * P],
                                rhs=qt_sb[d0 : d0 + D, mo, :],
                                start=True,
                                stop=True,
                            )
                    e_tiles = {}
                    for h in heads:
                        e = etmp.tile(
                            [P, 2, 512], BF16, tag="e", name=f"e_{h}_{t}", bufs=6
                        )
                        e_tiles[h] = e
                        nc.scalar.activation(out=e, in_=s_tiles[h], func=AF.Exp)
                    if t == 0 and after_first_exp is not None:
                        after_first_exp()
                    for h in heads:
                        nc.vector.tensor_tensor(
                            e_tiles[h],
                            e_tiles[h],
                            ebt_sb[:, h * 8 + 2 * t : h * 8 + 2 * t + 2, :],
                            mybir.AluOpType.mult,
                        )
                    if pre_pv is not None:
                        pre_pv(t)
                    for h in heads:
                        for j in range(2):
                            c = 2 * t + j
                            nc.tensor.matmul(
                                pv_ps[h],
                                lhsT=vaug_sb[
                                    :, c, h * (D + 1) : (h + 1) * (D + 1)
                                ],
                                rhs=e_tiles[h][:, j, :],
                                start=(c == 0),
                                stop=(c == 7),
                            )
                return pv_ps

            def oraw_evac(pv_ps):
                # one DVE copy per head grabs o rows AND the den row.
                # On DVE (not ACT): an ACT-resident copy head-of-line
                # blocks the next pair's exps behind PV completion.
                for h, ps in pv_ps.items():
                    nc.vector.tensor_copy(out=oraw_sb[:, h, :], in_=ps)

            def norm_batch(h0, cnt=4):
                """Normalize heads h0..h0+cnt (one DVE reciprocal for all)."""
                den4 = singles.tile([cnt, NQ], BF16, name=f"den4_{h0}")
                nc.sync.dma_start(
                    out=den4, in_=oraw_sb[D : D + 1, h0 : h0 + cnt, :]
                )
                recip4 = singles.tile([cnt, NQ], BF16, name=f"recip4_{h0}")
                with nc.allow_low_precision(
                    reason="softmax denom recip in bf16"
                ):
                    nc.vector.reciprocal(out=recip4, in_=den4)
                for i in range(cnt):
                    h = h0 + i
                    rbc_ps = ps_sm.tile([D, NQ], F32, tag="ps", name=f"rbc_{h}")
                    nc.tensor.matmul(
                        rbc_ps,
                        lhsT=sel8_sb[0:cnt, i * D : (i + 1) * D],
                        rhs=recip4,
                        start=True,
                        stop=True,
                    )
                    rbc_sb = vtmp.tile([D, NQ], BF16, tag="rbc", name=f"rbc_sb_{h}", bufs=4)
                    nc.scalar.copy(out=rbc_sb, in_=rbc_ps)
                    nc.vector.tensor_tensor(
                        osc_sb[:, h, :],
                        oraw_sb[0:D, h, :],
                        rbc_sb,
                        mybir.AluOpType.mult,
                    )
                # relocate: even heads -> partitions 0-63, odd -> 64-127
                mo0 = h0 // 2
                nmo = cnt // 2
                evens = osc_sb[:, h0 : h0 + cnt, :].rearrange(
                    "p (m t) q -> p m t q", t=2
                )
                nc.sync.dma_start(
                    out=ofin_sb[0:D, mo0 : mo0 + nmo, :], in_=evens[:, :, 0, :]
                )
                nc.sync.dma_start(
                    out=ofin_sb[D:P, mo0 : mo0 + nmo, :], in_=evens[:, :, 1, :]
                )

            tmpo_sb = singles.tile([P, 4, NQ], F32)  # to_out ko 0-1 partial
            gate2_sb = singles.tile([P, 4, NQ], BF16)

            def toout_p1(mo):
                """to_out over ko 0-1 (heads 0-3; ready after reloc of 0-3),
                staged to SBUF with bo folded in."""
                ps = ps_sm.tile([P, 512], F32, tag="ps", name="ps_o1")
                for ko in range(2):
                    nc.tensor.matmul(
                        ps,
                        lhsT=wot_sb[:, ko, mo * P : (mo + 1) * P],
                        rhs=ofin_sb[:, ko, :],
                        start=(ko == 0),
                        stop=(ko == 1),
                    )
                nc.vector.tensor_scalar_add(
                    out=tmpo_sb[:, mo, :], in0=ps, scalar1=bo_sb[:, mo : mo + 1]
                )

            def toout_p2(mo):
                ps = ps_sm.tile([P, 512], F32, tag="ps", name="ps_o2")
                for ko in range(2, 4):
                    nc.tensor.matmul(
                        ps,
                        lhsT=wot_sb[:, ko, mo * P : (mo + 1) * P],
                        rhs=ofin_sb[:, ko, :],
                        start=(ko == 2),
                        stop=(ko == 3),
                    )
                tmp = etmp.tile([P, NQ], F32, tag="otmp", name="otmp")
                nc.vector.tensor_add(out=tmp, in0=ps, in1=tmpo_sb[:, mo, :])
                nc.vector.tensor_tensor(
                    outf_sb[:, mo, :],
                    tmp,
                    gate2_sb[:, mo, :],
                    mybir.AluOpType.mult,
                )
                nc.sync.dma_start(out=out_e[:, mo, :], in_=outf_sb[:, mo, :])

            # schedule (v6 phasing): pair 0 early; bulk projections dense;
            # gate + to_out in the tail
            kt_proj(0)
            qt_proj(0)
            for c in range(8):
                v_proj(c)
            for mo in range(4):
                gate_proj(mo)
            nc.scalar.activation(out=gate_sb, in_=z_sb, func=AF.Tanh, scale=0.5)
            nc.vector.tensor_scalar(
                out=gate2_sb,
                in0=gate_sb,
                scalar1=0.5,
                scalar2=0.5,
                op0=mybir.AluOpType.mult,
                op1=mybir.AluOpType.add,
            )
            pv0 = attention_pair(0)
            oraw_evac(pv0)
            for mo in range(1, 4):
                kt_proj(mo)
                qt_proj(mo)
            pv1 = attention_pair(1)
            oraw_evac(pv1)
            norm_batch(0, 4)
            pv2 = attention_pair(2)
            oraw_evac(pv2)
            pv3 = attention_pair(3)
            oraw_evac(pv3)
            norm_batch(4, 4)
            for mo in range(4):
                toout_p1(mo)
            for mo in range(4):
                toout_p2(mo)

    nc.compile()
    return nc


def make_in_maps(q_x, attn_bias, Wq, bq, Wk, Wv, Wo, bo, Wg, bg, gating_bias):
    import ml_dtypes

    bf16 = ml_dtypes.bfloat16
    scale = np.float32(D) ** -0.5

    def swz(a2d):
        """[512, M] -> [128, 4, M] SBUF layout (partition-inner on dim 0)."""
        m = a2d.shape[1]
        return np.ascontiguousarray(a2d.reshape(4, P, m).transpose(1, 0, 2))

    wqt = swz(Wq.T.astype(np.float32) * scale).astype(bf16)
    wkt = swz(np.asarray(Wk.T, dtype=np.float32)).astype(bf16)
    wvt = swz(np.asarray(Wv.T, dtype=np.float32)).astype(bf16)
    wot = swz(np.asarray(Wo.T, dtype=np.float32)).astype(bf16)
    wgt = swz(np.asarray(Wg.T, dtype=np.float32)).astype(bf16)
    bqs = np.ascontiguousarray((bq * scale).reshape(4, P).T).astype(np.float32)
    bo_ = np.ascontiguousarray(np.asarray(bo).reshape(4, P).T).astype(np.float32)
    gb = np.ascontiguousarray((bg + gating_bias).reshape(4, P).T).astype(np.float32)
    sel8 = np.repeat(np.eye(H, dtype=np.float32), D, axis=1).astype(bf16)

    in_maps = []
    for c in range(8):
        b, half = c // 2, c % 2
        rows = slice(half * NQ, (half + 1) * NQ)
        x = np.asarray(q_x[b], dtype=np.float32)  # [N, CQ]
        xt = swz(x.T).astype(bf16)  # [128, 4, N]
        xtq = swz(np.ascontiguousarray(x[rows].T)).astype(bf16)
        # ebt[p, h*8+c, q] = exp(bias[b, h, rows, :]).T[c*128+p, q]
        eb = np.exp(np.asarray(attn_bias[b, :, rows, :], dtype=np.float32))
        ebt = np.ascontiguousarray(
            eb.transpose(0, 2, 1).reshape(H, 8, P, NQ).transpose(2, 0, 1, 3)
        ).reshape(P, H * 8, NQ).astype(bf16)
        in_maps.append(
            {
                "xt": xt,
                "xtq": xtq,
                "ebt": ebt,
                "wqt": wqt,
                "wkt": wkt,
                "wvt": wvt,
                "wot": wot,
                "wgt": wgt,
                "bqs": bqs,
                "bo": bo_,
                "gb": gb,
                "sel8": sel8,
            }
        )
    return in_maps


_NC_CACHE = None


def kernel(**inputs) -> np.ndarray:
    global _NC_CACHE
    from concourse.bass_utils import run_bass_kernel_spmd

    if _NC_CACHE is None:
        _NC_CACHE = build_nc()
    nc = _NC_CACHE
    in_maps = make_in_maps(**inputs)
    trace = bool(int(os.environ.get("BASS_KERNEL_TRACE", "0")))
    last_exc = None
    for attempt in range(3):
        try:
            res = run_bass_kernel_spmd(nc, in_maps, list(range(8)), trace=trace)
            break
        except Exception as exc:  # transient NRT/axon device hiccups
            last_exc = exc
            import time

            time.sleep(10 * (attempt + 1))
    else:
        raise last_exc
    kernel.last_result = res
    out = np.empty((B, N, CQ), dtype=np.float32)
    for c in range(8):
        b, half = c // 2, c % 2
        # res "out" is [128, 4, NQ]: out^T[cq=o*128+i, q] at [i, o, q]
        o = res.results[c]["out"]
        out[b, half * NQ : (half + 1) * NQ, :] = (
            o.transpose(1, 0, 2).reshape(CQ, NQ).T
        )
    return out


# revision 38
# speedup vs baseline: 1.0027x; 1.0027x over previous
"""Trainium2 Bass kernel for nn_Attention_79164837199973.

Bias-augmented multi-head self-attention with sigmoid gating.
B=4, N=1024, CQ=CH=512, H=8, D=64.

Sharding (8 cores, no collectives): core c -> batch b=c//2, query-row half
r=c%2 (512 rows). Each core computes k/v projections for the full sequence
of its batch (duplicated across the 2 cores of a batch pair -- cheaper than
an all-reduce), attention for all 8 heads over its 512 query rows, then
to_out + gating. Per-core outputs are exact disjoint shards of the result.

Layout notes (all matmuls bf16, f32 PSUM accumulate):
  - every DRAM input is host-pre-swizzled to its exact SBUF layout
    ([128 partitions, ...] with contiguous per-partition bytes) so each
    dma_start lowers to ~16 fat descriptors instead of ~1000 strided ones.
  - scores computed transposed: sT[nk,nq] via matmul(lhsT=kT[d,nk_chunk],
    rhs=qT[d,nq]); K=d=64, two heads row-packed into the PE array
    (partitions 0-63 / 64-127 concurrently).
  - softmax without max-subtraction (logits bounded ~+-7 here):
    p = exp(qk) * exp(bias), exp(bias) precomputed on host (bf16),
    multiplied in on VectorE (bf16 packed mode).
  - denominator via a ones-column appended to V (M=65 PV matmul, row 64);
    reciprocals batched 4 heads at a time into [4,512] DVE reciprocals
    (reciprocal costs ~6.5 cyc per per-lane element regardless of lane
    count, so 8 separate [1,512] ops would cost 8x).
  - per-head 1/denom broadcast over 64 partitions via tiny K=8 PE matmuls
    against a [8, 8*64] one-hot selector.
"""

import os
import sys

sys.path.insert(0, "/opt/trn_rl_repo")

import numpy as np

import concourse.bass as bass
import concourse.tile as tile
from concourse import bacc, mybir

B, N, CQ, CH, H = 4, 1024, 512, 512, 8
D = CH // H  # 64
NQ = N // 2  # 512 query rows per core
P = 128
F32 = mybir.dt.float32
BF16 = mybir.dt.bfloat16
AF = mybir.ActivationFunctionType


def build_nc():
    nc = bacc.Bacc("TRN2", target_bir_lowering=False, debug=False, num_devices=8)

    # ---- DRAM parameters, already in SBUF layout (host pre-swizzled) ----
    xt_e = nc.declare_dram_parameter("xt", [P, 4, N], BF16, isOutput=False)
    xtq_e = nc.declare_dram_parameter("xtq", [P, 4, NQ], BF16, isOutput=False)
    ebt_e = nc.declare_dram_parameter("ebt", [P, H * 8, NQ], BF16, isOutput=False)
    wqt_e = nc.declare_dram_parameter("wqt", [P, 4, CH], BF16, isOutput=False)
    wkt_e = nc.declare_dram_parameter("wkt", [P, 4, CH], BF16, isOutput=False)
    wvt_e = nc.declare_dram_parameter("wvt", [P, 4, CH], BF16, isOutput=False)
    wot_e = nc.declare_dram_parameter("wot", [P, 4, CQ], BF16, isOutput=False)
    wgt_e = nc.declare_dram_parameter("wgt", [P, 4, CQ], BF16, isOutput=False)
    bqs_e = nc.declare_dram_parameter("bqs", [P, 4], F32, isOutput=False)
    bo_e = nc.declare_dram_parameter("bo", [P, 4], F32, isOutput=False)
    gb_e = nc.declare_dram_parameter("gb", [P, 4], F32, isOutput=False)
    sel8_e = nc.declare_dram_parameter("sel8", [H, H * D], BF16, isOutput=False)
    out_e = nc.declare_dram_parameter("out", [P, 4, NQ], F32, isOutput=True)

    with tile.TileContext(nc) as tc:
        with (
            tc.tile_pool(name="singles", bufs=1) as singles,
            tc.tile_pool(name="etmp", bufs=3) as etmp,
            tc.tile_pool(name="vtmp", bufs=2) as vtmp,
            tc.tile_pool(name="ps_s", bufs=2, space="PSUM") as ps_s,
            tc.tile_pool(name="ps_sm", bufs=4, space="PSUM") as ps_sm,
        ):
            # ---- persistent SBUF tiles ----
            xt_sb = singles.tile([P, 4, N], BF16)
            xtq_sb = singles.tile([P, 4, NQ], BF16)
            wqt_sb = singles.tile([P, 4, CH], BF16)
            wkt_sb = singles.tile([P, 4, CH], BF16)
            wvt_sb = singles.tile([P, 4, CH], BF16)
            wot_sb = singles.tile([P, 4, CQ], BF16)
            wgt_sb = singles.tile([P, 4, CQ], BF16)
            bqs_sb = singles.tile([P, 4], F32)
            bo_sb = singles.tile([P, 4], F32)
            gb_sb = singles.tile([P, 4], F32)
            sel8_sb = singles.tile([H, H * D], BF16)
            ebt_sb = singles.tile([P, H * 8, NQ], BF16)
            kt_sb = singles.tile([P, 4, N], BF16)
            qt_sb = singles.tile([P, 4, NQ], BF16)
            vaug_sb = singles.tile([P, 8, H * (D + 1)], BF16)
            oraw_sb = singles.tile([D + 1, H, NQ], BF16)  # 0-63 o, row 64 den
            osc_sb = singles.tile([D, H, NQ], BF16)  # normalized o, all heads
            ofin_sb = singles.tile([P, 4, NQ], BF16)  # head-merged o^T
            z_sb = singles.tile([P, 4, NQ], F32)  # gate pre-activation
            gate_sb = singles.tile([P, 4, NQ], BF16)
            outf_sb = singles.tile([P, 4, NQ], F32)
            warm_sb = singles.tile([1, 8], F32)
            warmo_sb = singles.tile([1, 8], BF16)

            # force the exp table load off the critical path (first ACT op)
            nc.vector.memset(warm_sb, 0.0)
            nc.scalar.activation(out=warmo_sb, in_=warm_sb, func=AF.Exp)

            # ---- input DMAs: one HWDGE ring; FIFO order = priority order
            def ebt_load(h):
                nc.sync.dma_start(
                    out=ebt_sb[:, h * 8 : (h + 1) * 8, :],
                    in_=ebt_e[:, h * 8 : (h + 1) * 8, :],
                )

            nc.sync.dma_start(out=xt_sb, in_=xt_e[:, :, :])
            nc.sync.dma_start(out=wkt_sb, in_=wkt_e[:, :, :])
            nc.sync.dma_start(out=wqt_sb, in_=wqt_e[:, :, :])
            nc.sync.dma_start(out=xtq_sb, in_=xtq_e[:, :, :])
            nc.sync.dma_start(out=bqs_sb, in_=bqs_e[:, :])
            nc.sync.dma_start(out=wvt_sb, in_=wvt_e[:, :, :])
            for h in range(4):
                ebt_load(h)
            nc.sync.dma_start(out=wgt_sb, in_=wgt_e[:, :, :])
            nc.sync.dma_start(out=gb_sb, in_=gb_e[:, :])
            nc.sync.dma_start(out=sel8_sb, in_=sel8_e[:, :])
            for h in range(4, H):
                ebt_load(h)
            nc.sync.dma_start(out=wot_sb, in_=wot_e[:, :, :])
            nc.sync.dma_start(out=bo_sb, in_=bo_e[:, :])

            # ones column of v_aug (col D of each head's 65-wide group)
            nc.vector.memset(
                vaug_sb.rearrange("p c (h e) -> p c h e", h=H)[:, :, :, D : D + 1],
                1.0,
            )

            def kt_proj(mo, act_evac=False):
                for no in range(2):
                    ps = ps_sm.tile([P, 512], F32, tag="ps", name="ps_k")
                    for ko in range(4):
                        nc.tensor.matmul(
                            ps,
                            lhsT=wkt_sb[:, ko, mo * P : (mo + 1) * P],
                            rhs=xt_sb[:, ko, no * 512 : (no + 1) * 512],
                            start=(ko == 0),
                            stop=(ko == 3),
                        )
                    dst = kt_sb[:, mo, no * 512 : (no + 1) * 512]
                    if act_evac:
                        nc.scalar.copy(out=dst, in_=ps)
                    else:
                        nc.vector.tensor_copy(out=dst, in_=ps)

            def qt_proj(mo, act_evac=False):
                ps = ps_sm.tile([P, 512], F32, tag="ps", name="ps_q")
                for ko in range(4):
                    nc.tensor.matmul(
                        ps,
                        lhsT=wqt_sb[:, ko, mo * P : (mo + 1) * P],
                        rhs=xtq_sb[:, ko, :],
                        start=(ko == 0),
                        stop=(ko == 3),
                    )
                if act_evac:
                    nc.scalar.activation(
                        out=qt_sb[:, mo, :],
                        in_=ps,
                        func=AF.Identity,
                        bias=bqs_sb[:, mo : mo + 1],
                    )
                else:
                    nc.vector.tensor_scalar_add(
                        out=qt_sb[:, mo, :], in0=ps, scalar1=bqs_sb[:, mo : mo + 1]
                    )

            def v_proj(c, act_evac=False):
                ps = ps_sm.tile([P, 512], F32, tag="ps", name="ps_v")
                for ko in range(4):
                    nc.tensor.matmul(
                        ps,
                        lhsT=xt_sb[:, ko, c * P : (c + 1) * P],
                        rhs=wvt_sb[:, ko, :],
                        start=(ko == 0),
                        stop=(ko == 3),
                    )
                dst = vaug_sb.rearrange("p c (h e) -> p c h e", h=H)[:, c, :, 0:D]
                srcv = ps.rearrange("p (h d) -> p h d", h=H)
                if act_evac:
                    nc.scalar.copy(out=dst, in_=srcv)
                else:
                    nc.vector.tensor_copy(out=dst, in_=srcv)

            def gate_proj(mo):
                ps = ps_sm.tile([P, 512], F32, tag="ps", name="ps_g")
                for ko in range(4):
                    nc.tensor.matmul(
                        ps,
                        lhsT=wgt_sb[:, ko, mo * P : (mo + 1) * P],
                        rhs=xtq_sb[:, ko, :],
                        start=(ko == 0),
                        stop=(ko == 3),
                    )
                nc.vector.tensor_scalar_add(
                    out=z_sb[:, mo, :], in0=ps, scalar1=gb_sb[:, mo : mo + 1]
                )

            def attention_pair(hp, pre_pv=None, after_first_exp=None):
                heads = (2 * hp, 2 * hp + 1)
                pv_ps = {}
                for h in heads:
                    pv_ps[h] = ps_sm.tile(
                        [D + 1, NQ], F32, tag="ps", name=f"pv_{h}"
                    )
                for t in range(4):  # two nk-chunks of 128 per step
                    s_tiles = {}
                    for h in heads:
                        s_tiles[h] = ps_s.tile(
                            [P, 2, 512], F32, tag="s", name=f"s_{h}_{t}"
                        )
                    # j outer / h inner: the two heads' K=64 matmuls are
                    # adjacent in the PE stream -> row-packed concurrency
                    for j in range(2):
                        c = 2 * t + j
                        for h in heads:
                            d0 = (h % 2) * D
                            mo = h // 2
                            nc.tensor.matmul(
                                s_tiles[h][:, j, :],
                                lhsT=kt_sb[d0 : d0 + D, mo, c * P : (c + 1) * P],
                                rhs=qt_sb[d0 : d0 + D, mo, :],
                                start=True,
                                stop=True,
                            )
                    e_tiles = {}
                    for h in heads:
                        e = etmp.tile(
                            [P, 2, 512], BF16, tag="e", name=f"e_{h}_{t}", bufs=6
                        )
                        e_tiles[h] = e
                        nc.scalar.activation(out=e, in_=s_tiles[h], func=AF.Exp)
                    if t == 0 and after_first_exp is not None:
                        after_first_exp()
                    for h in heads:
                        nc.vector.tensor_tensor(
                            e_tiles[h],
                            e_tiles[h],
                            ebt_sb[:, h * 8 + 2 * t : h * 8 + 2 * t + 2, :],
                            mybir.AluOpType.mult,
                        )
                    if pre_pv is not None:
                        pre_pv(t)
                    for h in heads:
                        for j in range(2):
                            c = 2 * t + j
                            nc.tensor.matmul(
                                pv_ps[h],
                                lhsT=vaug_sb[
                                    :, c, h * (D + 1) : (h + 1) * (D + 1)
                                ],
                                rhs=e_tiles[h][:, j, :],
                                start=(c == 0),
                                stop=(c == 7),
                            )
                return pv_ps

            def oraw_evac(pv_ps):
                # one DVE copy per head grabs o rows AND the den row.
                # On DVE (not ACT): an ACT-resident copy head-of-line
                # blocks the next pair's exps behind PV completion.
                for h, ps in pv_ps.items():
                    nc.vector.tensor_copy(out=oraw_sb[:, h, :], in_=ps)

            def norm_batch(h0, cnt=4):
                """Normalize heads h0..h0+cnt (one DVE reciprocal for all)."""
                den4 = singles.tile([cnt, NQ], BF16, name=f"den4_{h0}")
                nc.sync.dma_start(
                    out=den4, in_=oraw_sb[D : D + 1, h0 : h0 + cnt, :]
                )
                recip4 = singles.tile([cnt, NQ], BF16, name=f"recip4_{h0}")
                with nc.allow_low_precision(
                    reason="softmax denom recip in bf16"
                ):
                    nc.vector.reciprocal(out=recip4, in_=den4)
                for i in range(cnt):
                    h = h0 + i
                    rbc_ps = ps_sm.tile([D, NQ], F32, tag="ps", name=f"rbc_{h}")
                    nc.tensor.matmul(
                        rbc_ps,
                        lhsT=sel8_sb[0:cnt, i * D : (i + 1) * D],
                        rhs=recip4,
                        start=True,
                        stop=True,
                    )
                    rbc_sb = vtmp.tile([D, NQ], BF16, tag="rbc", name=f"rbc_sb_{h}", bufs=4)
                    nc.scalar.copy(out=rbc_sb, in_=rbc_ps)
                    nc.vector.tensor_tensor(
                        osc_sb[:, h, :],
                        oraw_sb[0:D, h, :],
                        rbc_sb,
                        mybir.AluOpType.mult,
                    )
                # relocate: even heads -> partitions 0-63, odd -> 64-127
                mo0 = h0 // 2
                nmo = cnt // 2
                evens = osc_sb[:, h0 : h0 + cnt, :].rearrange(
                    "p (m t) q -> p m t q", t=2
                )
                nc.sync.dma_start(
                    out=ofin_sb[0:D, mo0 : mo0 + nmo, :], in_=evens[:, :, 0, :]
                )
                nc.sync.dma_start(
                    out=ofin_sb[D:P, mo0 : mo0 + nmo, :], in_=evens[:, :, 1, :]
                )

            tmpo_sb = singles.tile([P, 4, NQ], F32)  # to_out ko 0-1 partial
            gate2_sb = singles.tile([P, 4, NQ], BF16)

            def toout_p1(mo):
                """to_out over ko 0-1 (heads 0-3; ready after reloc of 0-3),
                staged to SBUF with bo folded in."""
                ps = ps_sm.tile([P, 512], F32, tag="ps", name="ps_o1")
                for ko in range(2):
                    nc.tensor.matmul(
                        ps,
                        lhsT=wot_sb[:, ko, mo * P : (mo + 1) * P],
                        rhs=ofin_sb[:, ko, :],
                        start=(ko == 0),
                        stop=(ko == 1),
                    )
                nc.vector.tensor_scalar_add(
                    out=tmpo_sb[:, mo, :], in0=ps, scalar1=bo_sb[:, mo : mo + 1]
                )

            def toout_p2(mo):
                ps = ps_sm.tile([P, 512], F32, tag="ps", name="ps_o2")
                for ko in range(2, 4):
                    nc.tensor.matmul(
                        ps,
                        lhsT=wot_sb[:, ko, mo * P : (mo + 1) * P],
                        rhs=ofin_sb[:, ko, :],
                        start=(ko == 2),
                        stop=(ko == 3),
                    )
                tmp = etmp.tile([P, NQ], F32, tag="otmp", name="otmp")
                nc.vector.tensor_add(out=tmp, in0=ps, in1=tmpo_sb[:, mo, :])
                nc.vector.tensor_tensor(
                    outf_sb[:, mo, :],
                    tmp,
                    gate2_sb[:, mo, :],
                    mybir.AluOpType.mult,
                )
                nc.sync.dma_start(out=out_e[:, mo, :], in_=outf_sb[:, mo, :])

            # schedule (v6 phasing): pair 0 early; bulk projections dense;
            # gate + to_out in the tail
            kt_proj(0)
            qt_proj(0)
            for c in range(8):
                v_proj(c)
            for mo in range(4):
                gate_proj(mo)
            pv0 = attention_pair(0)
            oraw_evac(pv0)
            for mo in range(1, 4):
                kt_proj(mo)
                qt_proj(mo)
            pv1 = attention_pair(1)
            oraw_evac(pv1)
            norm_batch(0, 4)
            pv2 = attention_pair(2)
            oraw_evac(pv2)
            pv3 = attention_pair(3)
            oraw_evac(pv3)
            norm_batch(4, 4)
            nc.scalar.activation(out=gate_sb, in_=z_sb, func=AF.Tanh, scale=0.5)
            nc.vector.tensor_scalar(
                out=gate2_sb,
                in0=gate_sb,
                scalar1=0.5,
                scalar2=0.5,
                op0=mybir.AluOpType.mult,
                op1=mybir.AluOpType.add,
            )
            for mo in range(4):
                toout_p1(mo)
            for mo in range(4):
                toout_p2(mo)

    nc.compile()
    return nc


def make_in_maps(q_x, attn_bias, Wq, bq, Wk, Wv, Wo, bo, Wg, bg, gating_bias):
    import ml_dtypes

    bf16 = ml_dtypes.bfloat16
    scale = np.float32(D) ** -0.5

    def swz(a2d):
        """[512, M] -> [128, 4, M] SBUF layout (partition-inner on dim 0)."""
        m = a2d.shape[1]
        return np.ascontiguousarray(a2d.reshape(4, P, m).transpose(1, 0, 2))

    wqt = swz(Wq.T.astype(np.float32) * scale).astype(bf16)
    wkt = swz(np.asarray(Wk.T, dtype=np.float32)).astype(bf16)
    wvt = swz(np.asarray(Wv.T, dtype=np.float32)).astype(bf16)
    wot = swz(np.asarray(Wo.T, dtype=np.float32)).astype(bf16)
    wgt = swz(np.asarray(Wg.T, dtype=np.float32)).astype(bf16)
    bqs = np.ascontiguousarray((bq * scale).reshape(4, P).T).astype(np.float32)
    bo_ = np.ascontiguousarray(np.asarray(bo).reshape(4, P).T).astype(np.float32)
    gb = np.ascontiguousarray((bg + gating_bias).reshape(4, P).T).astype(np.float32)
    sel8 = np.repeat(np.eye(H, dtype=np.float32), D, axis=1).astype(bf16)

    in_maps = []
    for c in range(8):
        b, half = c // 2, c % 2
        rows = slice(half * NQ, (half + 1) * NQ)
        x = np.asarray(q_x[b], dtype=np.float32)  # [N, CQ]
        xt = swz(x.T).astype(bf16)  # [128, 4, N]
        xtq = swz(np.ascontiguousarray(x[rows].T)).astype(bf16)
        # ebt[p, h*8+c, q] = exp(bias[b, h, rows, :]).T[c*128+p, q]
        eb = np.exp(np.asarray(attn_bias[b, :, rows, :], dtype=np.float32))
        ebt = np.ascontiguousarray(
            eb.transpose(0, 2, 1).reshape(H, 8, P, NQ).transpose(2, 0, 1, 3)
        ).reshape(P, H * 8, NQ).astype(bf16)
        in_maps.append(
            {
                "xt": xt,
                "xtq": xtq,
                "ebt": ebt,
                "wqt": wqt,
                "wkt": wkt,
                "wvt": wvt,
                "wot": wot,
                "wgt": wgt,
                "bqs": bqs,
                "bo": bo_,
                "gb": gb,
                "sel8": sel8,
            }
        )
    return in_maps


_NC_CACHE = None


def kernel(**inputs) -> np.ndarray:
    global _NC_CACHE
    from concourse.bass_utils import run_bass_kernel_spmd

    if _NC_CACHE is None:
        _NC_CACHE = build_nc()
    nc = _NC_CACHE
    in_maps = make_in_maps(**inputs)
    trace = bool(int(os.environ.get("BASS_KERNEL_TRACE", "0")))
    last_exc = None
    for attempt in range(3):
        try:
            res = run_bass_kernel_spmd(nc, in_maps, list(range(8)), trace=trace)
            break
        except Exception as exc:  # transient NRT/axon device hiccups
            last_exc = exc
            import time

            time.sleep(10 * (attempt + 1))
    else:
        raise last_exc
    kernel.last_result = res
    out = np.empty((B, N, CQ), dtype=np.float32)
    for c in range(8):
        b, half = c // 2, c % 2
        # res "out" is [128, 4, NQ]: out^T[cq=o*128+i, q] at [i, o, q]
        o = res.results[c]["out"]
        out[b, half * NQ : (half + 1) * NQ, :] = (
            o.transpose(1, 0, 2).reshape(CQ, NQ).T
        )
    return out
